# revision 9
# baseline (speedup 1.0000x reference)
"""Trainium2 Bass kernel for retrieval_knn (nn_DIONEMA_18021682774612).

Problem: per-token L2-normalize x, squared-L2 distances to 512 normalized
centroids, argmin + top-2 margin mask, masked per-cluster segment sums of the
raw features, plus counts.

Sharding: data-parallel over the batch axis — each of the 8 cores handles 8 of
the 64 batches (32768 tokens). Partial cluster_sums are reduced on the host
(tiny: 8 x 256KB); counts come from device-computed idx+mask via bincount.

Device algorithm per 128-token tile:
  - scores r = x_raw . c_hat (PE matmul, lhsT = d-major x tile, rhs = c_hatT)
    argmax_k(r) == argmin_k dist since dist_k = |x_hat|^2 + |c_hat_k|^2 - 2 r_k/|x|
    (per-row constant shift + positive scale; the ~1e-7 variation of |c_hat_k|^2
    is below fp32 matmul noise).
  - DVE max -> top-8 values; max_index -> argmax index (first occurrence,
    matching argmin tie-break). gap = 2*(r1-r2)/|x| > margin => above mask.
  - onehot[tok,k] = (iota==idx)*above (DVE tensor_scalar, one pass)
  - cluster_sumsT += x_tokmajor.T @ onehot (PE matmul accumulating in PSUM)
  - x_hat = x * rsqrt-style scale on ACT engine, written token-major; host
    restores (b,d,h,w).
"""

import os
import sys

sys.path.insert(0, "/opt/trn_rl_repo")

import numpy as np

import concourse.bass as bass
import concourse.bacc as bacc
import concourse.mybir as mybir
from concourse.tile import TileContext
from concourse.bass_utils import run_bass_kernel_spmd

# Problem dims (hardcoded per contest rules)
B, D, H, W = 64, 128, 64, 64
HW = H * W
K = 512
NCORES = 8
BPC = B // NCORES  # batches per core
NPC = BPC * HW  # tokens per core = 32768
P = 128
TILES = NPC // P  # 256
TPB = HW // P  # tiles per batch = 32
GR = 8  # tiles per stats group
NGROUPS = TILES // GR
MARGIN = 0.005
EPS = 1e-12

F32 = mybir.dt.float32
U32 = mybir.dt.uint32
ALU = mybir.AluOpType
ACTF = mybir.ActivationFunctionType

# Set True once verified that DVE max/max_index can read PSUM directly.
SCORES_VIA_SBUF = True


def _build_nc():
    nc = bacc.Bacc(
        "TRN2", target_bir_lowering=False, debug=False, num_devices=NCORES
    )
    xt = nc.dram_tensor("xt", (NPC, D), F32, kind="ExternalInput").ap()
    xd = nc.dram_tensor("xd", (BPC, D, HW), F32, kind="ExternalInput").ap()
    ct = nc.dram_tensor("ct", (D, K), F32, kind="ExternalInput").ap()
    iota = nc.dram_tensor("iota", (P, K), F32, kind="ExternalInput").ap()

    outt = nc.dram_tensor("outt", (NPC, D), F32, kind="ExternalOutput").ap()
    csum = nc.dram_tensor("csum", (D, K), F32, kind="ExternalOutput").ap()
    idxo = nc.dram_tensor("idxo", (P, TILES), F32, kind="ExternalOutput").ap()
    abvo = nc.dram_tensor("abvo", (P, TILES), F32, kind="ExternalOutput").ap()

    with (
        TileContext(nc) as tc,
        tc.tile_pool(name="const", bufs=1) as const_pool,
        tc.tile_pool(name="xt", bufs=18) as xt_pool,
        tc.tile_pool(name="xd", bufs=4) as xd_pool,
        tc.tile_pool(name="sq", bufs=2) as sq_pool,
        tc.tile_pool(name="ssb", bufs=4) as ssb_pool,
        tc.tile_pool(name="oh", bufs=3) as oh_pool,
        tc.tile_pool(name="xhat", bufs=4) as xhat_pool,
        tc.tile_pool(name="stat", bufs=3) as stat_pool,
        tc.tile_pool(name="psc", bufs=4, space="PSUM") as psc_pool,
        tc.tile_pool(name="pacc", bufs=1, space="PSUM") as pacc_pool,
    ):
        ct_sb = const_pool.tile([D, K], F32)
        nc.sync.dma_start(ct_sb, ct)
        iota_sb = const_pool.tile([P, K], F32)
        nc.sync.dma_start(iota_sb, iota)

        idx_stage = const_pool.tile([P, TILES], F32)
        abv_stage = const_pool.tile([P, TILES], F32)

        csum_ps = pacc_pool.tile([D, K], F32)

        for g in range(NGROUPS):
            ssq_g = stat_pool.tile([P, GR], F32, tag="ssq")
            top8_g = stat_pool.tile([P, 8 * GR], F32, tag="top8")
            tidx_g = stat_pool.tile([P, 8 * GR], U32, tag="tidx")
            xt_tiles = []
            for m in range(GR):
                t = g * GR + m
                b, j = divmod(t, TPB)
                xt_t = xt_pool.tile([P, D], F32, tag="xt")
                nc.sync.dma_start(xt_t, xt[t * P : (t + 1) * P, :])
                xd_t = xd_pool.tile([P, P], F32, tag="xd")
                nc.sync.dma_start(xd_t, xd[b, :, j * P : (j + 1) * P])

                # per-token sum of squares (ACT: square, accumulate over free)
                sq_t = sq_pool.tile([P, D], F32, tag="sq")
                nc.scalar.activation(
                    sq_t, xt_t, ACTF.Square, accum_out=ssq_g[:, m : m + 1]
                )

                # raw scores r = xd_t.T @ ct  -> (128 tok, 512 k) in PSUM
                sc_ps = psc_pool.tile([P, K], F32, tag="sc")
                nc.tensor.matmul(sc_ps, xd_t, ct_sb, start=True, stop=True)

                if SCORES_VIA_SBUF:
                    s_sb = ssb_pool.tile([P, K], F32, tag="ssb")
                    nc.scalar.activation(s_sb, sc_ps, ACTF.Copy)
                    s_src = s_sb
                else:
                    s_src = sc_ps

                nc.vector.max(out=top8_g[:, 8 * m : 8 * m + 8], in_=s_src)
                nc.vector.max_index(
                    out=tidx_g[:, 8 * m : 8 * m + 8],
                    in_max=top8_g[:, 8 * m : 8 * m + 8],
                    in_values=s_src,
                )
                xt_tiles.append(xt_t)

            # group stats: norm, clamp, inverse
            norm_g = stat_pool.tile([P, GR], F32, tag="norm")
            nc.scalar.activation(norm_g, ssq_g, ACTF.Sqrt)
            nc.vector.tensor_scalar_max(norm_g, norm_g, EPS)
            inv_g = stat_pool.tile([P, GR], F32, tag="inv")
            nc.vector.reciprocal(inv_g, norm_g)

            # gap = (r1 - r2) / |x|  ;  above = gap > margin/2
            diff_g = stat_pool.tile([P, GR], F32, tag="diff")
            nc.vector.tensor_sub(diff_g, top8_g[:, 0::8], top8_g[:, 1::8])
            nc.vector.tensor_mul(diff_g, diff_g, inv_g)
            nc.vector.tensor_scalar(
                abv_stage[:, g * GR : (g + 1) * GR],
                diff_g,
                MARGIN / 2.0,
                None,
                op0=ALU.is_gt,
            )
            # idx (top-1 indices), cast uint32 -> fp32
            nc.vector.tensor_copy(
                idx_stage[:, g * GR : (g + 1) * GR], tidx_g[:, 0::8]
            )

            # epilogue: onehot + cluster-sum matmul + normalized output
            for m in range(GR):
                t = g * GR + m
                oh = oh_pool.tile([P, K], F32, tag="oh")
                nc.vector.tensor_scalar(
                    oh,
                    iota_sb,
                    idx_stage[:, t : t + 1],
                    abv_stage[:, t : t + 1],
                    op0=ALU.is_equal,
                    op1=ALU.mult,
                )
                nc.tensor.matmul(
                    csum_ps,
                    xt_tiles[m],
                    oh,
                    start=(t == 0),
                    stop=(t == TILES - 1),
                )
                xhat = xhat_pool.tile([P, D], F32, tag="xhat")
                nc.scalar.activation(
                    xhat, xt_tiles[m], ACTF.Copy, scale=inv_g[:, m : m + 1]
                )
                nc.sync.dma_start(outt[t * P : (t + 1) * P, :], xhat)

        csum_sb = const_pool.tile([D, K], F32)
        nc.vector.tensor_copy(csum_sb, csum_ps)
        nc.sync.dma_start(csum, csum_sb)
        nc.sync.dma_start(idxo, idx_stage)
        nc.sync.dma_start(abvo, abv_stage)

    nc.compile()
    return nc


_NC = None
_LAST_RESULTS = None


def _register_ntff_shim():
    """The image's antenv lacks axon_hooks; register the NTFF profile hook
    directly from trn_agent_boot so trace=True works (dev/profiling only)."""
    import types

    if "antenv.axon_hooks" in sys.modules:
        return
    try:
        from trn_agent_boot.trn_boot import _ntff_profile_via_ctypes

        hook = _ntff_profile_via_ctypes("/opt/axon/libaxon_pjrt.so")
        mod = types.ModuleType("antenv.axon_hooks")
        mod.get_axon_ntff_profile_hook = lambda: hook
        mod.set_axon_ntff_profile_hook = lambda h: None
        sys.modules["antenv.axon_hooks"] = mod
    except Exception as e:  # profiling is best-effort
        print(f"ntff shim unavailable: {e}", flush=True)


def _get_nc():
    global _NC
    if _NC is None:
        _NC = _build_nc()
    return _NC


def kernel(x, centroid):
    x = np.ascontiguousarray(np.asarray(x, dtype=np.float32))
    centroid = np.ascontiguousarray(np.asarray(centroid, dtype=np.float32))

    # host-side input prep (layouts + centroid normalization, fp32 as reference)
    xt_full = np.ascontiguousarray(
        x.transpose(0, 2, 3, 1).reshape(B * HW, D)
    )  # token-major
    xd_full = x.reshape(B, D, HW)  # d-major (already contiguous)
    cn = np.sqrt(np.sum(centroid * centroid, axis=1, keepdims=True))
    c_hat = centroid / np.maximum(cn, EPS)
    ct = np.ascontiguousarray(c_hat.T)  # (128, 512)
    iota = np.ascontiguousarray(
        np.broadcast_to(np.arange(K, dtype=np.float32), (P, K))
    )

    nc = _get_nc()
    in_maps = [
        dict(
            xt=np.ascontiguousarray(xt_full[i * NPC : (i + 1) * NPC]),
            xd=np.ascontiguousarray(xd_full[i * BPC : (i + 1) * BPC]),
            ct=ct,
            iota=iota,
        )
        for i in range(NCORES)
    ]
    trace = bool(os.environ.get("KNN_TRACE"))
    if trace:
        _register_ntff_shim()
    try:
        res = run_bass_kernel_spmd(
            nc, in_maps, core_ids=list(range(NCORES)), trace=trace
        )
    except Exception:
        if not trace:
            raise
        import traceback

        traceback.print_exc()
        print("trace run failed; falling back to untraced run", flush=True)
        res = run_bass_kernel_spmd(nc, in_maps, core_ids=list(range(NCORES)))
    global _LAST_RESULTS
    _LAST_RESULTS = res
    results = res.results

    # host-side unshard
    out_tok = np.concatenate([r["outt"] for r in results], axis=0)
    out = np.ascontiguousarray(
        out_tok.reshape(B, H, W, D).transpose(0, 3, 1, 2)
    )

    csum_total = np.zeros((D, K), dtype=np.float64)
    for r in results:
        csum_total += r["csum"].astype(np.float64)
    cluster_sums = np.ascontiguousarray(csum_total.T.astype(np.float32))

    idx_parts = []
    abv_parts = []
    for r in results:
        idx_parts.append(
            np.rint(r["idxo"]).astype(np.int32).T.reshape(-1)
        )  # (P, TILES) -> token order
        abv_parts.append(r["abvo"].T.reshape(-1) > 0.5)
    idx = np.ascontiguousarray(np.concatenate(idx_parts))
    above = np.concatenate(abv_parts)
    counts = np.bincount(idx[above], minlength=K).astype(np.float32)

    return out, cluster_sums, counts, idx
